# revision 1
# baseline (speedup 1.0000x reference)
"""BCMSE loss kernel for 8 Trainium2 NeuronCores.

Strategy (pure data parallel, memory-bound):
  - Shard the [B, 9] inputs along batch across 8 cores (B/8 rows each).
  - Host-side, each shard is transposed to column-major [9, S] with rows
    permuted to [0,3,6,7,8,1,2,4,5] so that on device every column group
    (scalar {0,3}, vec {6,7,8}, angle {1,2,4,5}) is a contiguous block.
  - Each core streams its shard through SBUF in tiles of 128*q rows and
    reduces everything to 5 per-partition partial sums:
      p0   = sum (o-t)^2 over scalar cols
      p1   = sum wrapped-angle err^2 over angle cols
      p2   = sum (vmod-t)^2 over vec cols
      ext  = sum |floor(o)| over angle cols
      nrm  = sum norm over rows
  - Host combines 8x128 partials in float64 and applies the final formula.

Math notes:
  floor(x) = rne(x - 0.5) computed as (x + (1.5*2^23 - 0.5)) - 1.5*2^23 in
  one fused tensor_scalar op (exact except x exactly integral, measure-zero
  for randn inputs).
  angle |err| = min(|d|, ||d|-1|) with d = mod(o,1) - t  (algebraically equal
  to the reference's shortest-path target shift, incl. the |d|=0.5 boundary).
  vec remainder(v, norm) = v + norm*[v<0] exactly, because |v| <= norm.
"""
import numpy as np

import concourse.bacc as bacc
import concourse.mybir as mybir
from concourse.tile import TileContext
from concourse.bass_utils import run_bass_kernel_spmd

N_CORES = 8
BATCH = 4194304
SHARD = BATCH // N_CORES          # 524288 rows per core
P = 128
Q = 512                           # rows per partition per tile
TILE_ROWS = P * Q                 # 65536 rows per tile
N_TILES = SHARD // TILE_ROWS      # 8
PERM = [0, 3, 6, 7, 8, 1, 2, 4, 5]  # scalar(2) | vec(3) | angle(4)
MAGIC = float(1.5 * 2**23)        # rne magic for fp32
MAGIC_H = float(1.5 * 2**10)      # rne magic for fp16
HALF = True                       # ship fp16 to the device
CONSTANT_WEIGHT = 10.0

_cache = {}


def _build(shard, q, n_tiles, reps=1, mode='full', half=False):
    dt = mybir.dt.float16 if half else mybir.dt.float32
    magic = MAGIC_H if half else MAGIC
    f32 = mybir.dt.float32
    nc = bacc.Bacc("TRN2", target_bir_lowering=False)
    # host pre-tiles the data: row i*P+p holds tile i / partition p, 9q floats
    o_d = nc.dram_tensor("o", [n_tiles * P, 9 * q], dt, kind="ExternalInput")
    t_d = nc.dram_tensor("t", [n_tiles * P, 9 * q], dt, kind="ExternalInput")
    out_d = nc.dram_tensor("partials", [P, 8], f32, kind="ExternalOutput")

    with TileContext(nc) as tc:
        with (
            tc.tile_pool(name="io", bufs=3) as io,
            tc.tile_pool(name="scr", bufs=6) as scr,
            tc.tile_pool(name="acc", bufs=1) as acc,
        ):
            neg1 = acc.tile([P, 1], dt, tag="neg1")
            nc.vector.memset(neg1[:], -1.0)
            negM = acc.tile([P, 1], dt, tag="negM")
            nc.vector.memset(negM[:], -magic)
            s_p0 = acc.tile([P, n_tiles], f32, tag="s_p0")
            s_p1 = acc.tile([P, n_tiles], f32, tag="s_p1")
            s_p2 = acc.tile([P, n_tiles], f32, tag="s_p2")
            s_ext = acc.tile([P, n_tiles], f32, tag="s_ext")
            s_nrm = acc.tile([P, n_tiles], f32, tag="s_nrm")
            if mode == 'dma':
                for s in (s_p0, s_p1, s_p2, s_ext, s_nrm):
                    nc.vector.memset(s[:], 0.0)

            from contextlib import nullcontext
            loop = tc.For_i(0, reps, 1) if reps > 1 else nullcontext()
            with loop:
              for i in range(n_tiles):
                ot = io.tile([P, 9 * q], dt, tag="ot")
                tt = io.tile([P, 9 * q], dt, tag="tt")
                wid = 9 * q // 8 if mode == 'nodma' else 9 * q
                nc.sync.dma_start(out=ot[:, 0:wid], in_=o_d[i * P:(i + 1) * P, 0:wid])
                nc.sync.dma_start(out=tt[:, 0:wid], in_=t_d[i * P:(i + 1) * P, 0:wid])
                if mode == 'dma':
                    continue
                # contiguous column-group views (PERM order in DRAM)
                o_sc, t_sc = ot[:, 0:2 * q], tt[:, 0:2 * q]
                o_v, t_v = ot[:, 2 * q:5 * q], tt[:, 2 * q:5 * q]
                o_a, t_a = ot[:, 5 * q:9 * q], tt[:, 5 * q:9 * q]

                # ---- scalar cols: p0 += sum (o-t)^2
                pd = scr.tile([P, 2 * q], dt, tag="pd")
                nc.vector.tensor_sub(out=pd[:], in0=o_sc, in1=t_sc)
                nc.scalar.activation(out=pd[:], in_=pd[:],
                                     func=mybir.ActivationFunctionType.Square,
                                     accum_out=s_p0[:, i:i + 1])

                # ---- angle cols (all-DVE chain; ACT only for the two accums)
                # y = (o - 0.5) + magic; the fp16/fp32 output cast rounds at
                # ulp 1 in the magic range => y = floor(o) + magic
                y = scr.tile([P, 4 * q], dt, tag="y")
                nc.vector.tensor_scalar(out=y[:], in0=o_a,
                                        scalar1=0.5, scalar2=magic,
                                        op0=mybir.AluOpType.subtract,
                                        op1=mybir.AluOpType.add)
                fl = scr.tile([P, 4 * q], dt, tag="fl")
                nc.vector.tensor_scalar(out=fl[:], in0=y[:],
                                        scalar1=magic, scalar2=None,
                                        op0=mybir.AluOpType.subtract)
                nc.scalar.activation(out=y[:], in_=fl[:],
                                     func=mybir.ActivationFunctionType.Abs,
                                     accum_out=s_ext[:, i:i + 1])
                m = scr.tile([P, 4 * q], dt, tag="m")
                nc.vector.tensor_sub(out=m[:], in0=o_a, in1=fl[:])
                d = scr.tile([P, 4 * q], dt, tag="d")
                nc.vector.tensor_sub(out=d[:], in0=m[:], in1=t_a)
                # err = d - clamp(rne(d), -1, 1);  rne via fp32-stage magic
                nc.vector.tensor_scalar(out=m[:], in0=d[:],
                                        scalar1=MAGIC, scalar2=MAGIC,
                                        op0=mybir.AluOpType.add,
                                        op1=mybir.AluOpType.subtract)
                nc.vector.tensor_scalar(out=m[:], in0=m[:],
                                        scalar1=1.0, scalar2=-1.0,
                                        op0=mybir.AluOpType.min,
                                        op1=mybir.AluOpType.max)
                nc.vector.tensor_sub(out=d[:], in0=d[:], in1=m[:])
                nc.scalar.activation(out=d[:], in_=d[:],
                                     func=mybir.ActivationFunctionType.Square,
                                     accum_out=s_p1[:, i:i + 1])

                # ---- vec cols
                sq = scr.tile([P, 3 * q], dt, tag="sq")
                nc.scalar.activation(out=sq[:], in_=o_v,
                                     func=mybir.ActivationFunctionType.Square)
                nc.vector.tensor_add(out=sq[:, 0:q], in0=sq[:, 0:q], in1=sq[:, q:2 * q])
                nc.vector.tensor_add(out=sq[:, 0:q], in0=sq[:, 0:q], in1=sq[:, 2 * q:3 * q])
                nc.scalar.activation(out=sq[:, q:2 * q], in_=sq[:, 0:q],
                                     func=mybir.ActivationFunctionType.Sqrt,
                                     accum_out=s_nrm[:, i:i + 1])
                nrm = sq[:, q:2 * q]
                w = scr.tile([P, 3 * q], dt, tag="w")
                nc.vector.tensor_scalar(out=w[:], in0=o_v, scalar1=0.0,
                                        scalar2=None, op0=mybir.AluOpType.is_lt)
                for c in range(3):
                    nc.vector.tensor_mul(
                        out=w[:, c * q:(c + 1) * q], in0=w[:, c * q:(c + 1) * q],
                        in1=nrm)
                nc.vector.tensor_add(out=w[:], in0=o_v, in1=w[:])
                nc.vector.tensor_sub(out=w[:], in0=w[:], in1=t_v)
                nc.scalar.activation(out=w[:], in_=w[:],
                                     func=mybir.ActivationFunctionType.Square,
                                     accum_out=s_p2[:, i:i + 1])

            out_sb = acc.tile([P, 8], f32, tag="out_sb")
            nc.vector.memset(out_sb[:], 0.0)
            for j, s in enumerate([s_p0, s_p1, s_p2, s_ext, s_nrm]):
                nc.vector.tensor_reduce(out=out_sb[:, j:j + 1], in_=s[:],
                                        axis=mybir.AxisListType.X,
                                        op=mybir.AluOpType.add)
            nc.sync.dma_start(out=out_d[:], in_=out_sb[:])

    nc.compile()
    return nc


def _prep(arr, shard, core, q=Q, half=False):
    # [B, 9] row-major -> per-core pre-tiled [n_tiles*P, 9*q]:
    # row i*P+p = tile i / partition p, holding 9 blocks (PERM col order)
    # of q consecutive batch elements each
    sl = arr[core * shard:(core + 1) * shard, :]
    n_tiles = shard // (P * q)
    a = sl.reshape(n_tiles, P, q, 9).transpose(0, 1, 3, 2)[:, :, PERM, :]
    out = np.ascontiguousarray(a, dtype=np.float16 if half else np.float32)
    return out.reshape(n_tiles * P, 9 * q)


def _finish(partials, batch):
    # partials: [n_cores, 128, 8] fp32 -> final scalar, float64 combine
    tot = partials.astype(np.float64).sum(axis=(0, 1))
    p0, p1, p2, ext, nrm = tot[0], tot[1], tot[2], tot[3], tot[4]
    c0 = ext / batch / CONSTANT_WEIGHT
    c1 = nrm / batch / CONSTANT_WEIGHT
    mse = (p0 + p1 + p2) / (batch * 9)
    if (p0 > p1) and (p0 > p2):
        amount = 0.0
    elif (p0 > p1) and (p0 < p2):
        amount = c1
    elif (p0 < p1) and (p0 > p2):
        amount = c0
    else:
        amount = c0 + c1
    return np.float32(mse + amount)


def _run(outputs, targets, shard, q, n_tiles, n_cores, half=HALF, **spmd_kwargs):
    key = (shard, q, n_tiles, half)
    if key not in _cache:
        _cache[key] = _build(shard, q, n_tiles, half=half)
    nc = _cache[key]
    in_maps = [{"o": _prep(outputs, shard, k, q, half),
                "t": _prep(targets, shard, k, q, half)}
               for k in range(n_cores)]
    br = run_bass_kernel_spmd(nc, in_maps, list(range(n_cores)), **spmd_kwargs)
    partials = np.stack([r["partials"] for r in br.results])
    if spmd_kwargs:
        return partials, br
    return partials


def kernel(outputs, targets):
    outputs = np.asarray(outputs)
    targets = np.asarray(targets)
    assert outputs.shape == (BATCH, 9), outputs.shape
    partials = _run(outputs, targets, SHARD, Q, N_TILES, N_CORES)
    return _finish(partials, BATCH)

